# revision 5
# baseline (speedup 1.0000x reference)
"""Trainium2 Bass kernel for WeightedSignedConv (first_aggr=True) GCN block.

Strategy (8 NeuronCores, one SPMD program):
  - 50000 dst nodes are padded to 50176 = 392 tiles of 128; tiles are
    sorted by edge count and dealt to (core, slot) so all 8 cores see
    nearly identical work per slot (one shared program fits all cores).
  - Host-side: edges are bucketed by (dst chunk of 2 tiles, sign(edge_attr),
    src half), the 1/count normalization is folded into per-edge weights,
    buckets are padded to multiples of 128 using the max size across cores.
  - Device-side per core: gather x[src] rows via gpsimd.dma_gather, build a
    one-hot scatter matrix S[e, d] = w'_e * (dstloc_e == d) with one fused
    tensor_scalar, accumulate aggT[f, d] += Xg[e, f]^T S[e, d] on the
    tensor engine in PSUM (256-wide dst windows), then project
    out^T[o, d] = W_l^T agg + W_r^T x^T and finish with fused ReLU+bias.
  - Output is produced transposed ([256, D_core] per core); the host
    transposes/reorders, which is pure layout assembly.

The kernel is descriptor-generation bound (SWDGE ucode ~8ns/row), so the
whole data path runs in exact fp32 — the other engines hide underneath.
"""

import numpy as np

P = 128
NCORES = 8
CHUNK_TILES = 2          # dst tiles per PSUM window (256 dsts)
MSG_DT_NAME = "float32"  # gathered x + S dtype
PROJ_DT_NAME = "float32"  # projection matmul dtype
GATHER_MAX = 1024        # max idxs per dma_gather (descriptor ring cap)
DMA_SCRATCH = 16384      # SWDGE descriptor carveout bytes/partition


def _ceil_div(a, b):
    return (a + b - 1) // b


def _preprocess(x, src, dst, attr, slots_per_core, msg_np):
    """Bucket/pad edges; build per-core device arrays + block metadata."""
    n, f = x.shape
    assert f == P
    tiles_total = NCORES * slots_per_core
    n_pad = tiles_total * P
    half_rows = n_pad // 2

    pos = attr > 0
    neg = attr < 0
    keep = pos | neg
    absa = np.abs(attr)
    cntp = np.bincount(dst[pos], minlength=n).astype(np.float32)
    cntn = np.bincount(dst[neg], minlength=n).astype(np.float32)
    recp = 1.0 / np.maximum(cntp, 1.0)
    recn = 1.0 / np.maximum(cntn, 1.0)
    w1_all = absa.astype(np.float32) * np.where(pos, recp[dst], recn[dst])

    s_ = src[keep].astype(np.int64)
    d_ = dst[keep].astype(np.int64)
    sg = np.where(pos[keep], 0, 1).astype(np.int64)
    w1 = w1_all[keep].astype(np.float32)
    half = (s_ >= half_rows).astype(np.int64)

    tile_g = d_ // P

    # Sorted dealing: tile with edge-count rank r -> core r%8, slot r//8.
    tile_edges = np.bincount(tile_g, minlength=tiles_total)
    rank = np.argsort(np.argsort(-tile_edges))  # rank of each tile
    tile_core = rank % NCORES
    tile_slot = rank // NCORES

    core = tile_core[tile_g]
    slot = tile_slot[tile_g]
    chunk = slot // CHUNK_TILES
    dloc = (slot % CHUNK_TILES) * P + d_ % P  # dst index within chunk window

    n_chunks = _ceil_div(slots_per_core, CHUNK_TILES)

    # group key: (core, chunk, half, sign) — within a chunk the lo-half
    # section comes first (pos then neg), then the hi section.
    key = ((core * n_chunks + chunk) * 2 + half) * 2 + sg
    nkeys = NCORES * n_chunks * 4
    counts = np.bincount(key, minlength=nkeys).reshape(NCORES, n_chunks, 2, 2)
    blocks = _ceil_div(counts.max(axis=0), P)  # [chunk, half, sign]
    # every (chunk, sign) window needs >= 1 block for defined PSUM contents
    empty = blocks.sum(axis=1) == 0  # [chunk, sign]
    for c in range(n_chunks):
        for s in (0, 1):
            if empty[c, s]:
                blocks[c, 0, s] = 1

    # block layout + per-group start offsets
    gstart = np.zeros((n_chunks, 2, 2), dtype=np.int64)
    chunks = []  # (chunk_idx, width, chunk_block0, [(h, b0, nb)])
    b = 0
    for c in range(n_chunks):
        cb0 = b
        sections = []
        for h in (0, 1):
            h0 = b
            for s in (0, 1):
                gstart[c, h, s] = b
                b += int(blocks[c, h, s])
            sections.append((h, h0, b - h0))
        w = min(CHUNK_TILES, slots_per_core - c * CHUNK_TILES) * P
        chunks.append((c, w, cb0, sections))
    tot_blocks = b
    npad = tot_blocks * P

    # per-edge destination slot in the padded per-core arrays
    order = np.argsort(key, kind="stable")
    key_s = key[order]
    group_first = np.searchsorted(key_s, np.arange(nkeys), side="left")
    rank_e = np.arange(key_s.size) - group_first[key_s]
    gstart_flat = gstart.reshape(-1)
    local_key = key_s % (n_chunks * 4)
    eslot = gstart_flat[local_key] * P + rank_e

    core_s = key_s // (n_chunks * 4)
    srcloc_s = (s_ - half * half_rows)[order]
    dloc_s = dloc[order]
    w1_s = w1[order]

    idx16_list, dw_list, ww_list = [], [], []
    for c in range(NCORES):
        m = core_s == c
        sp = np.zeros(npad, dtype=np.int64)
        dp = np.zeros(npad, dtype=np.float64)
        wp = np.zeros(npad, dtype=np.float64)
        sp[eslot[m]] = srcloc_s[m]
        dp[eslot[m]] = dloc_s[m]
        wp[eslot[m]] = w1_s[m]
        tmp = sp.reshape(-1, 16).T.astype(np.int16)  # [16, npad/16]
        idx16_list.append(np.tile(tmp, (8, 1)))  # [128, npad/16]
        dw_list.append(np.ascontiguousarray(dp.reshape(-1, P).T).astype(msg_np))
        ww_list.append(np.ascontiguousarray(wp.reshape(-1, P).T).astype(msg_np))

    # window block lists: (chunk, sign) -> global block indices (lo then hi)
    windows = {}
    for c in range(n_chunks):
        for s in (0, 1):
            bl = list(range(gstart[c, 0, s], gstart[c, 0, s] + blocks[c, 0, s]))
            bl += list(range(gstart[c, 1, s], gstart[c, 1, s] + blocks[c, 1, s]))
            windows[(c, s)] = bl

    meta = dict(
        n=n,
        n_pad=n_pad,
        half_rows=half_rows,
        slots_per_core=slots_per_core,
        n_chunks=n_chunks,
        tot_blocks=tot_blocks,
        npad=npad,
        chunks=chunks,
        windows=windows,
        tile_core=tile_core,
        tile_slot=tile_slot,
    )
    return meta, idx16_list, dw_list, ww_list


def _build_program(meta, msg_dt, proj_dt):
    import concourse.bacc as bacc
    import concourse.mybir as mybir
    import concourse.tile as tile

    f32 = mybir.dt.float32
    dcore = meta["slots_per_core"] * P
    half_rows = meta["half_rows"]
    wmax = CHUNK_TILES * P

    nc = bacc.Bacc(
        "TRN2", target_bir_lowering=False, debug=False, num_devices=NCORES,
        dynamic_dma_scratch_size=DMA_SCRATCH,
    )
    xlo = nc.dram_tensor("xlo", [half_rows, P], msg_dt, kind="ExternalInput")
    xhi = nc.dram_tensor("xhi", [half_rows, P], msg_dt, kind="ExternalInput")
    idx16 = nc.dram_tensor(
        "idx16", [P, meta["npad"] // 16], mybir.dt.int16, kind="ExternalInput"
    )
    dlocd = nc.dram_tensor(
        "dloc", [P, meta["tot_blocks"]], msg_dt, kind="ExternalInput"
    )
    wpd = nc.dram_tensor(
        "wp", [P, meta["tot_blocks"]], msg_dt, kind="ExternalInput"
    )
    iotad = nc.dram_tensor("iota", [P, wmax], msg_dt, kind="ExternalInput")
    xTd = nc.dram_tensor("xT", [P, dcore], proj_dt, kind="ExternalInput")
    wd = {}
    for nm in ("wpl", "wpr", "wnl", "wnr"):
        wd[nm] = nc.dram_tensor(nm, [P, P], proj_dt, kind="ExternalInput")
    bd = {
        0: nc.dram_tensor("bpos", [P, 1], f32, kind="ExternalInput"),
        1: nc.dram_tensor("bneg", [P, 1], f32, kind="ExternalInput"),
    }
    outd = nc.dram_tensor("outT", [2 * P, dcore], f32, kind="ExternalOutput")

    with tile.TileContext(nc) as tc:
        with tc.tile_pool(name="const", bufs=1) as cpool, \
             tc.tile_pool(name="work", bufs=2) as wpool, \
             tc.tile_pool(name="spool", bufs=4) as spool, \
             tc.tile_pool(name="psum", bufs=2, space="PSUM") as ppool:
            idx_t = cpool.tile([P, meta["npad"] // 16], mybir.dt.int16)
            dloc_t = cpool.tile([P, meta["tot_blocks"]], msg_dt)
            wp_t = cpool.tile([P, meta["tot_blocks"]], msg_dt)
            iota_t = cpool.tile([P, wmax], msg_dt)
            w_t = {nm: cpool.tile([P, P], proj_dt, name=f"w_{nm}",
                                  tag=f"w_{nm}") for nm in wd}
            b_t = {s: cpool.tile([P, 1], f32, name=f"b_{s}", tag=f"b_{s}")
                   for s in (0, 1)}
            nc.sync.dma_start(out=idx_t[:], in_=idx16[:])
            nc.sync.dma_start(out=dloc_t[:], in_=dlocd[:])
            nc.sync.dma_start(out=wp_t[:], in_=wpd[:])
            nc.sync.dma_start(out=iota_t[:], in_=iotad[:])
            for nm in wd:
                nc.sync.dma_start(out=w_t[nm][:], in_=wd[nm][:])
            for s in (0, 1):
                nc.sync.dma_start(out=b_t[s][:], in_=bd[s][:])

            wl = {0: w_t["wpl"], 1: w_t["wnl"]}
            wr = {0: w_t["wpr"], 1: w_t["wnr"]}
            xsrc = {0: xlo, 1: xhi}

            for ci, w, cb0, sections in meta["chunks"]:
                nb_chunk = sum(nb for _, _, nb in sections)
                xg = wpool.tile([P, nb_chunk, P], msg_dt, name="xg", tag="xg")
                for h, b0, nbh in sections:
                    done = 0
                    while done < nbh:
                        g = min(nbh - done, GATHER_MAX // P)
                        gb0 = b0 + done
                        nc.gpsimd.dma_gather(
                            out_ap=xg[:, gb0 - cb0 : gb0 - cb0 + g, :],
                            in_ap=xsrc[h][:],
                            idxs_ap=idx_t[:, gb0 * 8 : (gb0 + g) * 8],
                            num_idxs=g * P,
                            num_idxs_reg=g * P,
                            elem_size=P,
                        )
                        done += g

                agg_ps = {
                    s: ppool.tile([P, w], f32, name=f"agg{s}", tag=f"agg{s}")
                    for s in (0, 1)
                }
                for s in (0, 1):
                    bl = meta["windows"][(ci, s)]
                    for j, gb in enumerate(bl):
                        s_t = spool.tile([P, w], msg_dt, name="S", tag="S")
                        nc.vector.tensor_scalar(
                            out=s_t[:],
                            in0=iota_t[:, :w],
                            scalar1=dloc_t[:, gb : gb + 1],
                            scalar2=wp_t[:, gb : gb + 1],
                            op0=mybir.AluOpType.is_equal,
                            op1=mybir.AluOpType.mult,
                        )
                        nc.tensor.matmul(
                            out=agg_ps[s][:],
                            lhsT=xg[:, gb - cb0, :],
                            rhs=s_t[:],
                            start=(j == 0),
                            stop=(j == len(bl) - 1),
                        )

                xT_t = wpool.tile([P, w], proj_dt, name="xT", tag="xT")
                nc.sync.dma_start(
                    out=xT_t[:],
                    in_=xTd[:, ci * wmax : ci * wmax + w],
                )
                for s in (0, 1):
                    agg_sb = wpool.tile([P, w], proj_dt, name=f"aggsb{s}",
                                        tag=f"aggsb{s}")
                    nc.vector.tensor_copy(out=agg_sb[:], in_=agg_ps[s][:])
                    out_ps = ppool.tile([P, w], f32, name=f"out{s}",
                                        tag=f"out{s}")
                    nc.tensor.matmul(
                        out=out_ps[:], lhsT=wl[s][:], rhs=agg_sb[:],
                        start=True, stop=False,
                    )
                    nc.tensor.matmul(
                        out=out_ps[:], lhsT=wr[s][:], rhs=xT_t[:],
                        start=False, stop=True,
                    )
                    out_sb = wpool.tile([P, w], f32, name=f"outsb{s}",
                                        tag=f"outsb{s}")
                    nc.scalar.activation(
                        out=out_sb[:], in_=out_ps[:],
                        func=mybir.ActivationFunctionType.Relu,
                        bias=b_t[s][:],
                    )
                    nc.sync.dma_start(
                        out=outd[s * P : (s + 1) * P,
                                 ci * wmax : ci * wmax + w],
                        in_=out_sb[:],
                    )
    nc.compile()
    return nc


def _run(x, edge_index, edge_attr, w_pos_l, w_pos_r, b_pos_r, w_neg_l,
         w_neg_r, b_neg_r, slots_per_core=49, sim=False, trace=False,
         trace_all=False):
    import concourse.mybir as mybir
    from concourse.bass_utils import run_bass_kernel_spmd

    msg_dt = getattr(mybir.dt, MSG_DT_NAME)
    proj_dt = getattr(mybir.dt, PROJ_DT_NAME)
    msg_np = np.dtype(mybir.dt.np(msg_dt))
    proj_np = np.float32  # float32r is float32 bits

    x = np.asarray(x, dtype=np.float32)
    edge_index = np.asarray(edge_index)
    edge_attr = np.asarray(edge_attr, dtype=np.float32)
    n, f = x.shape
    assert f == P

    meta, idx16_list, dw_list, ww_list = _preprocess(
        x, edge_index[0], edge_index[1], edge_attr, slots_per_core, msg_np
    )
    n_pad = meta["n_pad"]
    half_rows = meta["half_rows"]
    dcore = slots_per_core * P
    wmax = CHUNK_TILES * P

    xp = np.zeros((n_pad, P), dtype=np.float32)
    xp[:n] = x
    xlo = np.ascontiguousarray(xp[:half_rows]).astype(msg_np)
    xhi = np.ascontiguousarray(xp[half_rows:]).astype(msg_np)
    iota = np.tile(
        np.arange(wmax, dtype=np.float32)[None, :], (P, 1)
    ).astype(msg_np)

    weights = {
        "wpl": np.ascontiguousarray(np.asarray(w_pos_l, np.float32).T),
        "wpr": np.ascontiguousarray(np.asarray(w_pos_r, np.float32).T),
        "wnl": np.ascontiguousarray(np.asarray(w_neg_l, np.float32).T),
        "wnr": np.ascontiguousarray(np.asarray(w_neg_r, np.float32).T),
    }
    weights = {k: v.astype(proj_np) for k, v in weights.items()}
    bpos = np.asarray(b_pos_r, np.float32).reshape(P, 1)
    bneg = np.asarray(b_neg_r, np.float32).reshape(P, 1)

    nc = _build_program(meta, msg_dt, proj_dt)

    # per-core x shard in (core, slot) order, transposed
    tile_core, tile_slot = meta["tile_core"], meta["tile_slot"]
    xtiles = xp.reshape(-1, P, P)  # [tiles_total, 128, 128]
    in_maps = []
    for c in range(NCORES):
        mytiles = np.zeros((slots_per_core, P, P), dtype=np.float32)
        sel = tile_core == c
        mytiles[tile_slot[sel]] = xtiles[sel]
        xT_c = np.ascontiguousarray(
            mytiles.reshape(dcore, P).T
        ).astype(proj_np)
        in_maps.append(
            dict(
                xlo=xlo, xhi=xhi,
                idx16=idx16_list[c], dloc=dw_list[c], wp=ww_list[c],
                iota=iota, xT=xT_c,
                bpos=bpos, bneg=bneg, **weights,
            )
        )

    if sim:
        from concourse.bass_interp import MultiCoreSim

        ms = MultiCoreSim(nc, num_cores=NCORES)
        for c in range(NCORES):
            for name, arr in in_maps[c].items():
                ms.cores[c].tensor(name)[:] = arr
        ms.simulate()
        results = [
            {"outT": np.array(ms.cores[c].tensor("outT"))}
            for c in range(NCORES)
        ]
        exec_ns = None
    else:
        br = run_bass_kernel_spmd(
            nc, in_maps, list(range(NCORES)), trace=trace,
            trace_cores=list(range(NCORES)) if (trace and trace_all) else None,
        )
        results = br.results
        exec_ns = br.exec_time_ns

    # reassemble: core c slot k columns -> global tile rows
    out = np.empty((n_pad, 2 * P), dtype=np.float32)
    inv = np.empty(NCORES * slots_per_core, dtype=np.int64)
    inv[tile_core * slots_per_core + tile_slot] = np.arange(
        NCORES * slots_per_core
    )
    for c in range(NCORES):
        o = results[c]["outT"].T.reshape(slots_per_core, P, 2 * P)
        for k in range(slots_per_core):
            g = np.nonzero((tile_core == c) & (tile_slot == k))[0]
            if g.size:
                out[g[0] * P : g[0] * P + P] = o[k]
    return np.ascontiguousarray(out[:n]), exec_ns


def kernel(**inputs):
    out, _ = _run(**inputs)
    return out


# revision 6
# speedup vs baseline: 1.1392x; 1.1392x over previous
"""Trainium2 Bass kernel for WeightedSignedConv (first_aggr=True) GCN block.

Strategy (8 NeuronCores, one SPMD program):
  - 50000 dst nodes are padded to 50176 = 392 tiles of 128; tiles are
    sorted by edge count and dealt to (core, slot) so all 8 cores see
    nearly identical work per slot (one shared program fits all cores).
  - Host-side: edges are bucketed by (dst chunk of 2 tiles, sign(edge_attr),
    src half), the 1/count normalization is folded into per-edge weights,
    buckets are padded to multiples of 128 using the max size across cores.
  - Device-side per core: gather x[src] rows via gpsimd.dma_gather, build a
    one-hot scatter matrix S[e, d] = w'_e * (dstloc_e == d) with one fused
    tensor_scalar, accumulate aggT[f, d] += Xg[e, f]^T S[e, d] on the
    tensor engine in PSUM (256-wide dst windows), then project
    out^T[o, d] = W_l^T agg + W_r^T x^T and finish with fused ReLU+bias.
  - Output is produced transposed ([256, D_core] per core); the host
    transposes/reorders, which is pure layout assembly.

The kernel is descriptor-generation bound (SWDGE ucode ~8ns/row), so the
whole data path runs in exact fp32 — the other engines hide underneath.
"""

import numpy as np

P = 128
NCORES = 8
CHUNK_TILES = 2          # dst tiles per PSUM window (256 dsts)
MSG_DT_NAME = "float32"  # gathered x + S dtype
PROJ_DT_NAME = "float32"  # projection matmul dtype
GATHER_MAX = 1024        # max idxs per dma_gather (descriptor ring cap)
DMA_SCRATCH = 16384      # SWDGE descriptor carveout bytes/partition


def _ceil_div(a, b):
    return (a + b - 1) // b


def _preprocess(x, src, dst, attr, slots_per_core, msg_np):
    """Bucket/pad edges; build per-core device arrays + block metadata."""
    n, f = x.shape
    assert f == P
    tiles_total = NCORES * slots_per_core
    n_pad = tiles_total * P
    half_rows = n_pad // 2

    pos = attr > 0
    neg = attr < 0
    keep = pos | neg
    absa = np.abs(attr)
    cntp = np.bincount(dst[pos], minlength=n).astype(np.float32)
    cntn = np.bincount(dst[neg], minlength=n).astype(np.float32)
    recp = 1.0 / np.maximum(cntp, 1.0)
    recn = 1.0 / np.maximum(cntn, 1.0)
    w1_all = absa.astype(np.float32) * np.where(pos, recp[dst], recn[dst])

    s_ = src[keep].astype(np.int64)
    d_ = dst[keep].astype(np.int64)
    sg = np.where(pos[keep], 0, 1).astype(np.int64)
    w1 = w1_all[keep].astype(np.float32)
    half = (s_ >= half_rows).astype(np.int64)

    tile_g = d_ // P

    # Sorted dealing: tile with edge-count rank r -> core r%8, slot r//8.
    tile_edges = np.bincount(tile_g, minlength=tiles_total)
    rank = np.argsort(np.argsort(-tile_edges))  # rank of each tile
    tile_core = rank % NCORES
    tile_slot = rank // NCORES

    core = tile_core[tile_g]
    slot = tile_slot[tile_g]
    chunk = slot // CHUNK_TILES
    dloc = (slot % CHUNK_TILES) * P + d_ % P  # dst index within chunk window

    n_chunks = _ceil_div(slots_per_core, CHUNK_TILES)

    # group key: (core, chunk, half, sign) — within a chunk the lo-half
    # section comes first (pos then neg), then the hi section.
    key = ((core * n_chunks + chunk) * 2 + half) * 2 + sg
    nkeys = NCORES * n_chunks * 4
    counts = np.bincount(key, minlength=nkeys).reshape(NCORES, n_chunks, 2, 2)
    blocks = _ceil_div(counts.max(axis=0), P)  # [chunk, half, sign]
    # every (chunk, sign) window needs >= 1 block for defined PSUM contents
    empty = blocks.sum(axis=1) == 0  # [chunk, sign]
    for c in range(n_chunks):
        for s in (0, 1):
            if empty[c, s]:
                blocks[c, 0, s] = 1

    # block layout + per-group start offsets
    gstart = np.zeros((n_chunks, 2, 2), dtype=np.int64)
    chunks = []  # (chunk_idx, width, chunk_block0, [(h, b0, nb)])
    b = 0
    for c in range(n_chunks):
        cb0 = b
        sections = []
        for h in (0, 1):
            h0 = b
            for s in (0, 1):
                gstart[c, h, s] = b
                b += int(blocks[c, h, s])
            sections.append((h, h0, b - h0))
        w = min(CHUNK_TILES, slots_per_core - c * CHUNK_TILES) * P
        chunks.append((c, w, cb0, sections))
    tot_blocks = b
    npad = tot_blocks * P

    # per-edge destination slot in the padded per-core arrays
    order = np.argsort(key, kind="stable")
    key_s = key[order]
    group_first = np.searchsorted(key_s, np.arange(nkeys), side="left")
    rank_e = np.arange(key_s.size) - group_first[key_s]
    gstart_flat = gstart.reshape(-1)
    local_key = key_s % (n_chunks * 4)
    eslot = gstart_flat[local_key] * P + rank_e

    core_s = key_s // (n_chunks * 4)
    srcloc_s = (s_ - half * half_rows)[order]
    dloc_s = dloc[order]
    w1_s = w1[order]

    idx16_list, dw_list, ww_list = [], [], []
    for c in range(NCORES):
        m = core_s == c
        sp = np.zeros(npad, dtype=np.int64)
        dp = np.zeros(npad, dtype=np.float64)
        wp = np.zeros(npad, dtype=np.float64)
        sp[eslot[m]] = srcloc_s[m]
        dp[eslot[m]] = dloc_s[m]
        wp[eslot[m]] = w1_s[m]
        tmp = sp.reshape(-1, 16).T.astype(np.int16)  # [16, npad/16]
        idx16_list.append(np.tile(tmp, (8, 1)))  # [128, npad/16]
        dw_list.append(np.ascontiguousarray(dp.reshape(-1, P).T).astype(msg_np))
        ww_list.append(np.ascontiguousarray(wp.reshape(-1, P).T).astype(msg_np))

    # window block lists: (chunk, sign) -> global block indices (lo then hi)
    windows = {}
    for c in range(n_chunks):
        for s in (0, 1):
            bl = list(range(gstart[c, 0, s], gstart[c, 0, s] + blocks[c, 0, s]))
            bl += list(range(gstart[c, 1, s], gstart[c, 1, s] + blocks[c, 1, s]))
            windows[(c, s)] = bl

    meta = dict(
        n=n,
        n_pad=n_pad,
        half_rows=half_rows,
        slots_per_core=slots_per_core,
        n_chunks=n_chunks,
        tot_blocks=tot_blocks,
        npad=npad,
        chunks=chunks,
        windows=windows,
        tile_core=tile_core,
        tile_slot=tile_slot,
    )
    return meta, idx16_list, dw_list, ww_list


def _build_program(meta, msg_dt, proj_dt):
    import concourse.bacc as bacc
    import concourse.mybir as mybir
    import concourse.tile as tile

    f32 = mybir.dt.float32
    dcore = meta["slots_per_core"] * P
    half_rows = meta["half_rows"]
    wmax = CHUNK_TILES * P

    nc = bacc.Bacc(
        "TRN2", target_bir_lowering=False, debug=False, num_devices=NCORES,
        dynamic_dma_scratch_size=DMA_SCRATCH,
    )
    xlo = nc.dram_tensor("xlo", [half_rows, P], msg_dt, kind="ExternalInput")
    xhi = nc.dram_tensor("xhi", [half_rows, P], msg_dt, kind="ExternalInput")
    idx16 = nc.dram_tensor(
        "idx16", [P, meta["npad"] // 16], mybir.dt.int16, kind="ExternalInput"
    )
    dlocd = nc.dram_tensor(
        "dloc", [P, meta["tot_blocks"]], msg_dt, kind="ExternalInput"
    )
    wpd = nc.dram_tensor(
        "wp", [P, meta["tot_blocks"]], msg_dt, kind="ExternalInput"
    )
    iotad = nc.dram_tensor("iota", [P, wmax], msg_dt, kind="ExternalInput")
    xTd = nc.dram_tensor("xT", [P, dcore], proj_dt, kind="ExternalInput")
    wd = {}
    for nm in ("wpl", "wpr", "wnl", "wnr"):
        wd[nm] = nc.dram_tensor(nm, [P, P], proj_dt, kind="ExternalInput")
    bd = {
        0: nc.dram_tensor("bpos", [P, 1], f32, kind="ExternalInput"),
        1: nc.dram_tensor("bneg", [P, 1], f32, kind="ExternalInput"),
    }
    outd = nc.dram_tensor("outT", [2 * P, dcore], f32, kind="ExternalOutput")

    with tile.TileContext(nc) as tc:
        with tc.tile_pool(name="const", bufs=1) as cpool, \
             tc.tile_pool(name="work", bufs=3) as wpool, \
             tc.tile_pool(name="spool", bufs=6) as spool, \
             tc.tile_pool(name="psum", bufs=2, space="PSUM") as ppool:
            idx_t = cpool.tile([P, meta["npad"] // 16], mybir.dt.int16)
            dloc_t = cpool.tile([P, meta["tot_blocks"]], msg_dt)
            wp_t = cpool.tile([P, meta["tot_blocks"]], msg_dt)
            iota_t = cpool.tile([P, wmax], msg_dt)
            w_t = {nm: cpool.tile([P, P], proj_dt, name=f"w_{nm}",
                                  tag=f"w_{nm}") for nm in wd}
            b_t = {s: cpool.tile([P, 1], f32, name=f"b_{s}", tag=f"b_{s}")
                   for s in (0, 1)}
            nc.sync.dma_start(out=idx_t[:], in_=idx16[:])
            nc.sync.dma_start(out=dloc_t[:], in_=dlocd[:])
            nc.sync.dma_start(out=wp_t[:], in_=wpd[:])
            nc.sync.dma_start(out=iota_t[:], in_=iotad[:])
            for nm in wd:
                nc.sync.dma_start(out=w_t[nm][:], in_=wd[nm][:])
            for s in (0, 1):
                nc.sync.dma_start(out=b_t[s][:], in_=bd[s][:])

            wl = {0: w_t["wpl"], 1: w_t["wnl"]}
            wr = {0: w_t["wpr"], 1: w_t["wnr"]}
            xsrc = {0: xlo, 1: xhi}

            for ci, w, cb0, sections in meta["chunks"]:
                nb_chunk = sum(nb for _, _, nb in sections)
                xg = wpool.tile([P, nb_chunk, P], msg_dt, name="xg", tag="xg")
                for h, b0, nbh in sections:
                    done = 0
                    while done < nbh:
                        g = min(nbh - done, GATHER_MAX // P)
                        gb0 = b0 + done
                        nc.gpsimd.dma_gather(
                            out_ap=xg[:, gb0 - cb0 : gb0 - cb0 + g, :],
                            in_ap=xsrc[h][:],
                            idxs_ap=idx_t[:, gb0 * 8 : (gb0 + g) * 8],
                            num_idxs=g * P,
                            num_idxs_reg=g * P,
                            elem_size=P,
                            single_packet=False,
                        )
                        done += g

                agg_ps = {
                    s: ppool.tile([P, w], f32, name=f"agg{s}", tag=f"agg{s}")
                    for s in (0, 1)
                }
                for s in (0, 1):
                    bl = meta["windows"][(ci, s)]
                    for j, gb in enumerate(bl):
                        s_t = spool.tile([P, w], msg_dt, name="S", tag="S")
                        nc.vector.tensor_scalar(
                            out=s_t[:],
                            in0=iota_t[:, :w],
                            scalar1=dloc_t[:, gb : gb + 1],
                            scalar2=wp_t[:, gb : gb + 1],
                            op0=mybir.AluOpType.is_equal,
                            op1=mybir.AluOpType.mult,
                        )
                        nc.tensor.matmul(
                            out=agg_ps[s][:],
                            lhsT=xg[:, gb - cb0, :],
                            rhs=s_t[:],
                            start=(j == 0),
                            stop=(j == len(bl) - 1),
                        )

                xT_t = wpool.tile([P, w], proj_dt, name="xT", tag="xT")
                nc.sync.dma_start(
                    out=xT_t[:],
                    in_=xTd[:, ci * wmax : ci * wmax + w],
                )
                for s in (0, 1):
                    agg_sb = wpool.tile([P, w], proj_dt, name=f"aggsb{s}",
                                        tag=f"aggsb{s}")
                    nc.scalar.copy(out=agg_sb[:], in_=agg_ps[s][:])
                    out_ps = ppool.tile([P, w], f32, name=f"out{s}",
                                        tag=f"out{s}")
                    nc.tensor.matmul(
                        out=out_ps[:], lhsT=wl[s][:], rhs=agg_sb[:],
                        start=True, stop=False,
                    )
                    nc.tensor.matmul(
                        out=out_ps[:], lhsT=wr[s][:], rhs=xT_t[:],
                        start=False, stop=True,
                    )
                    out_sb = wpool.tile([P, w], f32, name=f"outsb{s}",
                                        tag=f"outsb{s}")
                    nc.scalar.activation(
                        out=out_sb[:], in_=out_ps[:],
                        func=mybir.ActivationFunctionType.Relu,
                        bias=b_t[s][:],
                    )
                    nc.sync.dma_start(
                        out=outd[s * P : (s + 1) * P,
                                 ci * wmax : ci * wmax + w],
                        in_=out_sb[:],
                    )
    nc.compile()
    return nc


def _run(x, edge_index, edge_attr, w_pos_l, w_pos_r, b_pos_r, w_neg_l,
         w_neg_r, b_neg_r, slots_per_core=49, sim=False, trace=False,
         trace_all=False):
    import concourse.mybir as mybir
    from concourse.bass_utils import run_bass_kernel_spmd

    msg_dt = getattr(mybir.dt, MSG_DT_NAME)
    proj_dt = getattr(mybir.dt, PROJ_DT_NAME)
    msg_np = np.dtype(mybir.dt.np(msg_dt))
    proj_np = np.float32  # float32r is float32 bits

    x = np.asarray(x, dtype=np.float32)
    edge_index = np.asarray(edge_index)
    edge_attr = np.asarray(edge_attr, dtype=np.float32)
    n, f = x.shape
    assert f == P

    meta, idx16_list, dw_list, ww_list = _preprocess(
        x, edge_index[0], edge_index[1], edge_attr, slots_per_core, msg_np
    )
    n_pad = meta["n_pad"]
    half_rows = meta["half_rows"]
    dcore = slots_per_core * P
    wmax = CHUNK_TILES * P

    xp = np.zeros((n_pad, P), dtype=np.float32)
    xp[:n] = x
    xlo = np.ascontiguousarray(xp[:half_rows]).astype(msg_np)
    xhi = np.ascontiguousarray(xp[half_rows:]).astype(msg_np)
    iota = np.tile(
        np.arange(wmax, dtype=np.float32)[None, :], (P, 1)
    ).astype(msg_np)

    weights = {
        "wpl": np.ascontiguousarray(np.asarray(w_pos_l, np.float32).T),
        "wpr": np.ascontiguousarray(np.asarray(w_pos_r, np.float32).T),
        "wnl": np.ascontiguousarray(np.asarray(w_neg_l, np.float32).T),
        "wnr": np.ascontiguousarray(np.asarray(w_neg_r, np.float32).T),
    }
    weights = {k: v.astype(proj_np) for k, v in weights.items()}
    bpos = np.asarray(b_pos_r, np.float32).reshape(P, 1)
    bneg = np.asarray(b_neg_r, np.float32).reshape(P, 1)

    nc = _build_program(meta, msg_dt, proj_dt)

    # per-core x shard in (core, slot) order, transposed
    tile_core, tile_slot = meta["tile_core"], meta["tile_slot"]
    xtiles = xp.reshape(-1, P, P)  # [tiles_total, 128, 128]
    in_maps = []
    for c in range(NCORES):
        mytiles = np.zeros((slots_per_core, P, P), dtype=np.float32)
        sel = tile_core == c
        mytiles[tile_slot[sel]] = xtiles[sel]
        xT_c = np.ascontiguousarray(
            mytiles.reshape(dcore, P).T
        ).astype(proj_np)
        in_maps.append(
            dict(
                xlo=xlo, xhi=xhi,
                idx16=idx16_list[c], dloc=dw_list[c], wp=ww_list[c],
                iota=iota, xT=xT_c,
                bpos=bpos, bneg=bneg, **weights,
            )
        )

    if sim:
        from concourse.bass_interp import MultiCoreSim

        ms = MultiCoreSim(nc, num_cores=NCORES)
        for c in range(NCORES):
            for name, arr in in_maps[c].items():
                ms.cores[c].tensor(name)[:] = arr
        ms.simulate()
        results = [
            {"outT": np.array(ms.cores[c].tensor("outT"))}
            for c in range(NCORES)
        ]
        exec_ns = None
    else:
        br = run_bass_kernel_spmd(
            nc, in_maps, list(range(NCORES)), trace=trace,
            trace_cores=list(range(NCORES)) if (trace and trace_all) else None,
        )
        results = br.results
        exec_ns = br.exec_time_ns

    # reassemble: core c slot k columns -> global tile rows
    out = np.empty((n_pad, 2 * P), dtype=np.float32)
    inv = np.empty(NCORES * slots_per_core, dtype=np.int64)
    inv[tile_core * slots_per_core + tile_slot] = np.arange(
        NCORES * slots_per_core
    )
    for c in range(NCORES):
        o = results[c]["outT"].T.reshape(slots_per_core, P, 2 * P)
        for k in range(slots_per_core):
            g = np.nonzero((tile_core == c) & (tile_slot == k))[0]
            if g.size:
                out[g[0] * P : g[0] * P + P] = o[k]
    return np.ascontiguousarray(out[:n]), exec_ns


def kernel(**inputs):
    out, _ = _run(**inputs)
    return out
